# revision 30
# baseline (speedup 1.0000x reference)
"""MultiHead PNCA attention on 8 trn2 NeuronCores.

Sharding: core c handles batch b = c // 4 and the 4 heads
[4*(c%4), 4*(c%4)+4).  Each core computes LN(x), its Q/K/V slices, both
attention branches (causal self-attn + memory cross-attn), writes its 4
heads of attn_x / attn_h, and a row-parallel partial of the output
projection.  The host sums the 4 partials per batch and adds the
residual and fc biases.

Self-contained: shapes hardcoded, no sibling imports.
"""

import os
import numpy as np

import concourse.bass as bass
import concourse.mybir as mybir
import concourse.tile as tile
from concourse.bass_utils import run_bass_kernel_spmd
from concourse.masks import make_identity
from concourse.vector_clock import ScopedClock

F32 = mybir.dt.float32
F32R = mybir.dt.float32r
AF = mybir.ActivationFunctionType
OP = mybir.AluOpType

B, LX, LH = 2, 2048, 1024
D = 1024            # d_model == d_mem
H, DH = 16, 64      # total heads, head dim
HL = 4              # heads per core
EPS = 1e-6
NEG = -1e30
P = 128

_NC_CACHE = {}


def _patch_tile_drain():
    """This walrus build allows only ONE sync-wait on a Drain instruction;
    Tile's exit attaches every outstanding sem wait to a single Drain.
    Split the waits onto individual nop carriers instead."""
    if getattr(tile.TileContext, "_drain_patched", False):
        return

    def _drain_and_barrier(self, tick_clock, wait_clock):
        nc = self.nc
        probe = nc.sync.nop()
        wait_clock.add_sem_waits(
            probe.ins, ScopedClock({None: tick_clock.global_clock})
        )
        waits = []
        if probe.ins.sync_info is not None and probe.ins.sync_info.on_wait:
            waits = list(probe.ins.sync_info.on_wait)
            probe.ins.sync_info.on_wait = []
        for w in waits:
            n = nc.sync.nop()
            n.ins.sync_info = mybir.SyncInfo(on_wait=[w], on_update=[])
        nc.sync.drain()
        nc.all_engine_barrier()
        assert self.sems is not None
        popped = nc._tile_sem_poison_stack.pop()
        assert popped is self._sem_poison
        nc.clear_and_free_semaphores(list(self.sems.allocated().values()))
        nc.all_engine_barrier()

    tile.TileContext._drain_and_barrier = _drain_and_barrier
    tile.TileContext._drain_patched = True


def build_nc():
    """One NeuronCore program (SPMD: all cores run this on different data)."""
    _patch_tile_drain()
    nc = bass.Bass("TRN2", target_bir_lowering=False, debug=False)

    dp = lambda name, shape: nc.declare_dram_parameter(name, shape, F32, isOutput=False)
    x_d = dp("x", [LX, D])
    h_d = dp("h", [LH, D])
    lnw_d = dp("ln_w", [D])
    lnb_d = dp("ln_b", [D])
    wqk_d = dp("w_qk", [D, 2 * HL * DH])     # [1024, 512]
    wv_d = dp("w_v", [D, HL * DH])           # [1024, 256]
    whk_d = dp("w_hk", [D, HL * DH])
    whv_d = dp("w_hv", [D, HL * DH])
    bqk_d = dp("b_qk", [2 * HL * DH])
    bv_d = dp("b_v", [HL * DH])
    bhk_d = dp("b_hk", [HL * DH])
    bhv_d = dp("b_hv", [HL * DH])
    wfcx_d = dp("w_fcx", [HL * DH, D])       # [256, 1024]
    wfch_d = dp("w_fch", [HL * DH, D])

    ax_d = nc.declare_dram_parameter("attn_x", [HL, LX, LX], F32, isOutput=True)
    ah_d = nc.declare_dram_parameter("attn_h", [HL, LX, LH], F32, isOutput=True)
    out_d = nc.declare_dram_parameter("out_part", [LX, D], F32, isOutput=True)

    with tile.TileContext(nc) as tc:
        _build_body(nc, tc, locals())
    _legalize_waits(nc)
    return nc


def _legalize_waits(nc, limit=1):
    """This walrus build accepts at most `limit` sync-waits per instruction.
    Move excess waits onto same-engine nop carriers inserted just before."""
    cnt = 0
    for fn in nc.m.functions:
        for blk in fn.blocks:
            out = []
            for ins in blk.instructions:
                si = ins.sync_info
                if si is not None and si.on_wait and len(si.on_wait) > limit:
                    waits = list(si.on_wait)
                    for w in waits[:-limit]:
                        nop = mybir.InstNoOp(name=f"I-wfix{cnt}", ins=[], outs=[])
                        cnt += 1
                        nop.engine = ins.engine
                        nop.sync_info = mybir.SyncInfo(on_wait=[w], on_update=[])
                        out.append(nop)
                    ins.sync_info = mybir.SyncInfo(
                        on_wait=waits[-limit:], on_update=list(si.on_update))
                out.append(ins)
            blk.instructions = out


def _build_body(nc, tc, d):
    x_d, h_d = d["x_d"], d["h_d"]
    ax_d, ah_d, out_d = d["ax_d"], d["ah_d"], d["out_d"]

    with (
        tc.tile_pool(name="consts", bufs=1) as consts,
        tc.tile_pool(name="persist", bufs=1) as persist,
        tc.tile_pool(name="dram", bufs=2, space="DRAM") as dram,
    ):
        # ---- constants ----
        ident = consts.tile([P, P], F32)
        make_identity(nc, ident)
        eps_t = consts.tile([P, 1], F32)
        nc.vector.memset(eps_t, EPS)
        lnw_t = consts.tile([P, 8], F32)
        nc.sync.dma_start(out=lnw_t, in_=d["lnw_d"].rearrange("(o p) -> p o", p=P))
        lnb_t = consts.tile([P, 8], F32)
        nc.sync.dma_start(out=lnb_t, in_=d["lnb_d"].rearrange("(o p) -> p o", p=P))
        bqk_t = consts.tile([P, 4], F32)
        nc.sync.dma_start(out=bqk_t, in_=d["bqk_d"].rearrange("(o p) -> p o", p=P))
        bhk_t = consts.tile([P, 2], F32)
        nc.sync.dma_start(out=bhk_t, in_=d["bhk_d"].rearrange("(o p) -> p o", p=P))

        def bcast_load(src_d, n):
            t = consts.tile([P, n], F32)
            src = src_d[:]
            ap = bass.AP(tensor=src.tensor, offset=src.offset,
                         ap=[[0, P]] + list(src.ap))
            nc.gpsimd.dma_start(out=t, in_=ap)
            return t

        bv_rep = bcast_load(d["bv_d"], HL * DH)      # [128, 256]
        bhv_rep = bcast_load(d["bhv_d"], HL * DH)

        # ---- persistent activations ----
        qkT = persist.tile([P, 4, LX], F32R)       # mt 0-1: q, 2-3: k; 2 heads/mt
        v_aug = persist.tile([P, 16, HL * 65], F32R)   # [j-tile, head*65 + (0..63 v, 64 ones)]
        hkT = persist.tile([P, 2, LH], F32R)
        hv_aug = persist.tile([P, 8, HL * 65], F32R)

        ones_t = consts.tile([P, 16], F32)
        nc.vector.memset(ones_t, 1.0)
        for hh in range(HL):
            nc.gpsimd.dma_start(out=v_aug[:, :, hh * 65 + 64], in_=ones_t[:, :16])
            nc.gpsimd.dma_start(out=hv_aug[:, :, hh * 65 + 64], in_=ones_t[:, :8])

        # ================= phase A: LN + projections =================
        _phase_x(nc, tc, d, qkT, v_aug, ident, eps_t, lnw_t, lnb_t, bqk_t, bv_rep)
        _phase_h(nc, tc, d, hkT, hv_aug, ident, bhk_t, bhv_rep)

        with tc.tile_pool(name="late", bufs=1) as late:
            otx = late.tile([P, 2, LX], F32R)      # normalized out_x^T  [hd', i]
            oth = late.tile([P, 2, LX], F32R)

            # ================= phase B: attention =================
            with (
                tc.tile_pool(name="ppo", bufs=1, space="PSUM") as ppo,
                tc.tile_pool(name="pss", bufs=2, space="PSUM") as pss,
                tc.tile_pool(name="sp2", bufs=2, space="PSUM") as sp2p,
                tc.tile_pool(name="psr", bufs=1, space="PSUM") as psrp,
                tc.tile_pool(name="et", bufs=4) as etp,
                tc.tile_pool(name="erow", bufs=4) as erp,
                tc.tile_pool(name="small", bufs=8) as smp,
                tc.tile_pool(name="rt", bufs=2) as rtp,
                tc.tile_pool(name="rb", bufs=1) as rbp,
            ):
                # interleave: a pass-1 is always traced between a pass-2
                # and the pass-1 it depends on, so its matmul/exp work hides
                # the rowsum -> recip_p latency.
                def mk(hh):
                    off = 64 * (hh % 2)
                    return dict(
                        qT=qkT[off:off + 64, hh // 2, :],
                        kT=qkT[off:off + 64, 2 + hh // 2, :],
                        hkT=hkT[off:off + 64, hh // 2, :],
                        va=v_aug[:, :, hh * 65:(hh + 1) * 65],
                        hva=hv_aug[:, :, hh * 65:(hh + 1) * 65],
                        otx=otx[off:off + 64, hh // 2, :],
                        oth=oth[off:off + 64, hh // 2, :])

                def p1(hh, br):
                    t = mk(hh)
                    if br == "x":
                        return _pass1(nc, tc, hh, t["qT"], t["kT"], t["va"],
                                      t["otx"], True, ppo, pss, psrp, etp,
                                      rtp, rbp, dram, ident)
                    return _pass1(nc, tc, hh, t["qT"], t["hkT"], t["hva"],
                                  t["oth"], False, ppo, pss, psrp, etp,
                                  rtp, rbp, dram, ident)

                def p2(hh, br, rp):
                    t = mk(hh)
                    if br == "x":
                        _pass2(nc, tc, hh, t["qT"], t["kT"], ax_d, True,
                               sp2p, erp, smp, rp)
                    else:
                        _pass2(nc, tc, hh, t["qT"], t["hkT"], ah_d, False,
                               sp2p, erp, smp, rp)

                rp = {}
                rp[(0, "x")] = p1(0, "x")
                rp[(0, "h")] = p1(0, "h")
                p2(0, "x", rp[(0, "x")])
                for hh in range(1, HL):
                    rp[(hh, "x")] = p1(hh, "x")
                    p2(hh - 1, "h", rp[(hh - 1, "h")])
                    rp[(hh, "h")] = p1(hh, "h")
                    p2(hh, "x", rp[(hh, "x")])
                p2(HL - 1, "h", rp[(HL - 1, "h")])

            # ================= phase C: output projection =================
            with (
                tc.tile_pool(name="pc", bufs=2, space="PSUM") as pcp,
                tc.tile_pool(name="co", bufs=3) as cop,
                tc.tile_pool(name="wfc", bufs=1) as wfcp,
            ):
                wfcx_t = wfcp.tile([P, 2, D], F32R)
                nc.gpsimd.dma_start(out=wfcx_t,
                                    in_=d["wfcx_d"].rearrange("(o p) m -> p o m", p=P))
                wfch_t = wfcp.tile([P, 2, D], F32R)
                nc.gpsimd.dma_start(out=wfch_t,
                                    in_=d["wfch_d"].rearrange("(o p) m -> p o m", p=P))
                for it in range(LX // P):
                    osb = cop.tile([P, D], F32)
                    for mc in range(2):
                        ps = pcp.tile([P, 512], F32)
                        for kt in range(2):
                            nc.tensor.matmul(
                                ps, otx[:, kt, it * P:(it + 1) * P],
                                wfcx_t[:, kt, mc * 512:(mc + 1) * 512],
                                start=(kt == 0), stop=False)
                        for kt in range(2):
                            nc.tensor.matmul(
                                ps, oth[:, kt, it * P:(it + 1) * P],
                                wfch_t[:, kt, mc * 512:(mc + 1) * 512],
                                start=False, stop=(kt == 1))
                        nc.vector.tensor_copy(out=osb[:, mc * 512:(mc + 1) * 512],
                                              in_=ps)
                    nc.sync.dma_start(out=out_d[it * P:(it + 1) * P, :], in_=osb)


def _phase_x(nc, tc, d, qkT, v_aug, ident, eps_t, lnw_t, lnb_t, bqk_t, bv_rep):
    with (
        tc.tile_pool(name="xin", bufs=3) as xin,
        tc.tile_pool(name="xnT", bufs=1) as xnTp,
        tc.tile_pool(name="wq", bufs=1) as wqp,
        tc.tile_pool(name="pa", bufs=1, space="PSUM") as pa,
        tc.tile_pool(name="pt", bufs=2, space="PSUM") as pt,
        tc.tile_pool(name="pv", bufs=2, space="PSUM") as pv,
        tc.tile_pool(name="stat", bufs=4) as statp,
    ):
        xnT = xnTp.tile([P, 8, LX], F32R)
        wqk_t = wqp.tile([P, 8, 512], F32R)
        nc.gpsimd.dma_start(out=wqk_t, in_=d["wqk_d"].rearrange("(o p) m -> p o m", p=P))
        wv_t = wqp.tile([P, 8, 256], F32R)
        nc.gpsimd.dma_start(out=wv_t, in_=d["wv_d"].rearrange("(o p) m -> p o m", p=P))

        # LN + transpose + projections, pipelined per 512-column seq chunk
        for sc in range(4):
            for st in range(4 * sc, 4 * sc + 4):
                xt = xin.tile([P, D], F32, name="xt", tag="xt")
                nc.sync.dma_start(out=xt, in_=d["x_d"][st * P:(st + 1) * P, :])
                stats = statp.tile([P, 2, nc.vector.BN_STATS_DIM], F32,
                                   name="stats", tag="stats")
                for sg in range(2):
                    nc.vector.bn_stats(out=stats[:, sg, :],
                                       in_=xt[:, sg * 512:(sg + 1) * 512])
                mv = statp.tile([P, 2], F32, name="mv", tag="mv")
                nc.vector.bn_aggr(out=mv, in_=stats)
                rstd = statp.tile([P, 1], F32, name="rstd", tag="rstd")
                nc.scalar.activation(out=rstd, in_=mv[:, 1:2], func=AF.Sqrt,
                                     bias=eps_t, scale=1.0)
                nc.vector.reciprocal(out=rstd, in_=rstd)
                nc.vector.tensor_scalar(out=xt, in0=xt, scalar1=mv[:, 0:1],
                                        scalar2=rstd, op0=OP.subtract, op1=OP.mult)
                for ct in range(8):
                    pst = pt.tile([P, P], F32, name="pst", tag="pst")
                    nc.tensor.transpose(pst, xt[:, ct * P:(ct + 1) * P], ident)
                    nc.vector.tensor_scalar(
                        out=xnT[:, ct, st * P:(st + 1) * P], in0=pst,
                        scalar1=lnw_t[:, ct:ct + 1], scalar2=lnb_t[:, ct:ct + 1],
                        op0=OP.mult, op1=OP.add)
            # q/k projection for this seq chunk
            for mt in range(4):
                ps = pa.tile([P, 512], F32, name=f"paq{mt}", tag=f"paq{mt}")
                for kt in range(8):
                    nc.tensor.matmul(
                        ps, wqk_t[:, kt, mt * P:(mt + 1) * P],
                        xnT[:, kt, sc * 512:(sc + 1) * 512],
                        start=(kt == 0), stop=(kt == 7))
                nc.vector.tensor_scalar(
                    out=qkT[:, mt, sc * 512:(sc + 1) * 512], in0=ps,
                    scalar1=bqk_t[:, mt:mt + 1], scalar2=None, op0=OP.add)
            # v projection for this seq chunk
            for st in range(4 * sc, 4 * sc + 4):
                psv = pv.tile([P, 256], F32, name="psv", tag="psv")
                for kt in range(8):
                    nc.tensor.matmul(psv, xnT[:, kt, st * P:(st + 1) * P],
                                     wv_t[:, kt, :], start=(kt == 0), stop=(kt == 7))
                for hh in range(HL):
                    nc.vector.tensor_tensor(
                        out=v_aug[:, st, hh * 65:hh * 65 + 64],
                        in0=psv[:, hh * 64:(hh + 1) * 64],
                        in1=bv_rep[:, hh * 64:(hh + 1) * 64], op=OP.add)


def _phase_h(nc, tc, d, hkT, hv_aug, ident, bhk_t, bhv_rep):
    with (
        tc.tile_pool(name="hin", bufs=3) as hin,
        tc.tile_pool(name="hT", bufs=1) as hTp,
        tc.tile_pool(name="wh", bufs=1) as whp,
        tc.tile_pool(name="pa2", bufs=1, space="PSUM") as pa,
        tc.tile_pool(name="pt2", bufs=2, space="PSUM") as pt,
        tc.tile_pool(name="pv2", bufs=2, space="PSUM") as pv,
    ):
        hT = hTp.tile([P, 8, LH], F32R)
        whk_t = whp.tile([P, 8, 256], F32R)
        nc.gpsimd.dma_start(out=whk_t, in_=d["whk_d"].rearrange("(o p) m -> p o m", p=P))
        whv_t = whp.tile([P, 8, 256], F32R)
        nc.gpsimd.dma_start(out=whv_t, in_=d["whv_d"].rearrange("(o p) m -> p o m", p=P))

        for st in range(LH // P):
            ht = hin.tile([P, D], F32)
            nc.sync.dma_start(out=ht, in_=d["h_d"][st * P:(st + 1) * P, :])
            for ct in range(8):
                pst = pt.tile([P, P], F32)
                nc.tensor.transpose(pst, ht[:, ct * P:(ct + 1) * P], ident)
                nc.vector.tensor_copy(out=hT[:, ct, st * P:(st + 1) * P], in_=pst)

        for mt in range(2):
            pss = [pa.tile([P, 512], F32, name=f'pah{sc}') for sc in range(2)]
            for kt in range(8):
                for sc in range(2):
                    nc.tensor.matmul(
                        pss[sc], whk_t[:, kt, mt * P:(mt + 1) * P],
                        hT[:, kt, sc * 512:(sc + 1) * 512],
                        start=(kt == 0), stop=(kt == 7))
            for sc in range(2):
                nc.vector.tensor_scalar(
                    out=hkT[:, mt, sc * 512:(sc + 1) * 512], in0=pss[sc],
                    scalar1=bhk_t[:, mt:mt + 1], scalar2=None, op0=OP.add)

        for st in range(LH // P):
            psv = pv.tile([P, 256], F32)
            for kt in range(8):
                nc.tensor.matmul(psv, hT[:, kt, st * P:(st + 1) * P],
                                 whv_t[:, kt, :], start=(kt == 0), stop=(kt == 7))
            for hh in range(HL):
                nc.vector.tensor_tensor(
                    out=hv_aug[:, st, hh * 65:hh * 65 + 64],
                    in0=psv[:, hh * 64:(hh + 1) * 64],
                    in1=bhv_rep[:, hh * 64:(hh + 1) * 64], op=OP.add)


def _pass1(nc, tc, hh, qT_h, kT_h, va_h, ot_h, causal,
           ppo, pss, psrp, etp, rtp, rbp, dram, ident):
    """S^T -> E^T -> unnormalized out^T + rowsum (via ones column), then
    normalize into ot_h [64, LX]."""
    lk = LX if causal else LH
    njt = lk // P
    pou = rtp.tile([65, LX], F32, name="pou", tag="pou")
    for ic in range(4):
        last_jt = (4 * ic + 3) if causal else (njt - 1)
        po = ppo.tile([65, 512], F32, name="po", tag="po")
        for jt in range(last_jt + 1):
            sp = pss.tile([P, 512], F32)
            nc.tensor.matmul(sp, kT_h[:, jt * P:(jt + 1) * P],
                             qT_h[:, ic * 512:(ic + 1) * 512],
                             start=True, stop=True)
            et = etp.tile([P, 512], F32R)
            nc.scalar.activation(out=et, in_=sp, func=AF.Exp, scale=0.125)
            if causal and jt // 4 == ic:
                # zero E^T where j > i: keep f - p - 128*(jt%4) >= 0
                nc.gpsimd.affine_select(
                    out=et, in_=et, compare_op=OP.is_ge, fill=0.0,
                    base=-128 * (jt % 4), channel_multiplier=-1,
                    pattern=[[1, 512]])
            nc.tensor.matmul(po, va_h[:, jt, :], et,
                             start=(jt == 0), stop=(jt == last_jt))
        # evacuate this i-chunk promptly (frees the PSUM bank)
        nc.vector.tensor_copy(out=pou[:, ic * 512:(ic + 1) * 512], in_=po)
    recipT = rtp.tile([1, LX], F32, name="recipT", tag="recipT")
    nc.vector.reciprocal(out=recipT, in_=pou[64:65, :])
    # per-partition layout [128, 16] of recipT via PE transposes (for pass 2)
    psr = psrp.tile([P, 16], F32, name="psr", tag="psr")
    for t in range(16):
        nc.tensor.transpose(psr[:, t:t + 1], recipT[:, t * P:(t + 1) * P], ident[0:1, 0:1])
    recip_p = rtp.tile([P, 16], F32, name="recip_p", tag="recip_p")
    nc.vector.tensor_copy(out=recip_p, in_=psr)
    # broadcast recipT over 64 partitions via a DRAM round-trip; normalize
    dscr = dram.tile([1, LX], F32)
    nc.sync.dma_start(out=dscr, in_=recipT)
    rb = rbp.tile([64, LX], F32)
    bap = bass.AP(tensor=dscr.tensor, offset=dscr.offset,
                  ap=[[0, 64]] + list(dscr.ap)[1:])
    nc.gpsimd.dma_start(out=rb, in_=bap)
    nc.vector.tensor_tensor(out=ot_h, in0=pou[0:64, :], in1=rb, op=OP.mult)
    return recip_p


def _pass2(nc, tc, hh, qT_h, kT_h, attn_d, causal, sp2, erp, smp, recip_p):
    """S -> P (normalized with pass-1 rowsums, masked) -> DRAM."""
    lk = LX if causal else LH
    for it in range(LX // P):
        njc = it // 4 + 1 if causal else lk // 512
        erow = erp.tile([P, LX if causal else LH], F32, name="erow", tag="erow")
        for c2 in range((njc + 1) // 2):
            w = min(1024, (njc - 2 * c2) * 512)   # 1024 or trailing 512
            sp = sp2.tile([P, 1024], F32, name="sp2", tag="sp2")
            for sub in range(w // 512):
                jc = 2 * c2 + sub
                nc.tensor.matmul(sp[:, sub * 512:(sub + 1) * 512],
                                 qT_h[:, it * P:(it + 1) * P],
                                 kT_h[:, jc * 512:(jc + 1) * 512],
                                 start=True, stop=True)
            lo = c2 * 1024
            nc.scalar.activation(out=erow[:, lo:lo + w], in_=sp[:, :w],
                                 func=AF.Exp, scale=0.125)
            nc.vector.tensor_scalar_mul(
                out=erow[:, lo:lo + w], in0=erow[:, lo:lo + w],
                scalar1=recip_p[:, it:it + 1])
        if causal:
            jc = it // 4
            # zero masked region (j > i): keep 128*(it%4) + p - f >= 0
            nc.gpsimd.affine_select(
                out=erow[:, jc * 512:(jc + 1) * 512],
                in_=erow[:, jc * 512:(jc + 1) * 512],
                compare_op=OP.is_ge, fill=0.0,
                base=128 * (it % 4), channel_multiplier=1,
                pattern=[[-1, 512]])
        if causal and njc * 512 < LX:
            nc.gpsimd.memset(erow[:, njc * 512:], 0.0)
        nc.sync.dma_start(out=attn_d[hh, it * P:(it + 1) * P, :], in_=erow)


# ======================= host side =======================

def _get_nc():
    if "nc" not in _NC_CACHE:
        _NC_CACHE["nc"] = build_nc()
    return _NC_CACHE["nc"]


def kernel(x, h, mask_x, mask_h, ln_w, ln_b, w_qkv, b_qkv, w_fcx, b_fcx,
           w_hkv, b_hkv, w_fch, b_fch):
    x = np.asarray(x, np.float32)
    h = np.asarray(h, np.float32)
    ln_w = np.asarray(ln_w, np.float32)
    ln_b = np.asarray(ln_b, np.float32)
    w_qkv = np.asarray(w_qkv, np.float32)
    b_qkv = np.asarray(b_qkv, np.float32)
    w_hkv = np.asarray(w_hkv, np.float32)
    b_hkv = np.asarray(b_hkv, np.float32)
    w_fcx = np.asarray(w_fcx, np.float32)
    w_fch = np.asarray(w_fch, np.float32)

    HD = H * DH
    wq, wk, wv = w_qkv[:, :HD], w_qkv[:, HD:2 * HD], w_qkv[:, 2 * HD:]
    bq, bk, bv = b_qkv[:HD], b_qkv[HD:2 * HD], b_qkv[2 * HD:]
    whk, whv = w_hkv[:, :HD], w_hkv[:, HD:]
    bhk, bhv = b_hkv[:HD], b_hkv[HD:]

    in_maps = []
    for c in range(8):
        b = c // 4
        g = c % 4
        cs = slice(g * HL * DH, (g + 1) * HL * DH)   # 256 cols for 4 heads
        in_maps.append({
            "x": np.ascontiguousarray(x[b]),
            "h": np.ascontiguousarray(h[b]),
            "ln_w": ln_w, "ln_b": ln_b,
            "w_qk": np.ascontiguousarray(np.concatenate([wq[:, cs], wk[:, cs]], axis=1)),
            "w_v": np.ascontiguousarray(wv[:, cs]),
            "w_hk": np.ascontiguousarray(whk[:, cs]),
            "w_hv": np.ascontiguousarray(whv[:, cs]),
            "b_qk": np.ascontiguousarray(np.concatenate([bq[cs], bk[cs]])),
            "b_v": np.ascontiguousarray(bv[cs]),
            "b_hk": np.ascontiguousarray(bhk[cs]),
            "b_hv": np.ascontiguousarray(bhv[cs]),
            "w_fcx": np.ascontiguousarray(w_fcx[cs, :]),
            "w_fch": np.ascontiguousarray(w_fch[cs, :]),
        })

    nc = _get_nc()
    trace = os.environ.get("KERNEL_TRACE", "0") == "1"
    if not trace:
        # NTFF profiling hooks (antenv.axon_hooks) are absent in this
        # container; make sure a stray BASS_TRACE can't crash the run.
        os.environ["BASS_NEVER_TRACE"] = "1"
    res = run_bass_kernel_spmd(nc, in_maps, list(range(8)), trace=trace)
    _NC_CACHE["last_res"] = res

    attn_x = np.empty((B, H, LX, LX), np.float32)
    attn_h = np.empty((B, H, LX, LH), np.float32)
    output = np.empty((B, LX, D), np.float32)
    for b in range(B):
        acc = x[b] + np.asarray(b_fcx, np.float32) + np.asarray(b_fch, np.float32)
        acc = acc.astype(np.float32)
        for g in range(4):
            r = res.results[4 * b + g]
            attn_x[b, g * HL:(g + 1) * HL] = r["attn_x"]
            attn_h[b, g * HL:(g + 1) * HL] = r["attn_h"]
            acc = acc + r["out_part"]
        output[b] = acc
    return output, attn_x, attn_h
